# revision 1
# baseline (speedup 1.0000x reference)
"""Trainium2 Bass kernel for nn_MultiHeadedSelfAttention_5179730559275.

Reference math (per batch b):
  q = wq @ x + bq ; k = wk @ x + bk ; v = wv @ x + bv        (1x1 conv, C=256 -> O=256)
  per o-channel (o = head*32 + d), with Q_o,K_o,V_o = 64x64 images [H,W]:
    S_o = Q_o @ K_o^T / sqrt(32); P_o = softmax(S_o, axis=-1); ctx_o = P_o @ V_o

Sharding: data-parallel over batch, 2 batches per core on 8 cores.

Per-core pipeline (per batch):
  1. fp32 x tiles [c, pix] -> PE projections (lhsT = w^T fp16 stationary,
     rhs = x as float32r moving, N=512) -> psum [o', 512]
  2. psum->SBUF copies add bias, cast fp16, and write interleaved layouts
     pairing o with o+128 (om = o chunk):
       q16/k16: [j, h*128 + om*64 + w]   (j = o mod 128)
       v16:     [j, w*128 + om*64 + g]
  3. xbar DMA transposes of 128x128 slices give matmul-ready layouts:
       qS/kS: [om*64 + w, h, j]  (per-o transposed images, o-pair stacked)
       vS:    [om*64 + g, w, j]  (natural images + ones column for Z)
  4. Attention per pair j: quadrant matmuls (K=64 at partition bases 0/64):
       S^T psum [om*64+g, h] ; exp (ACT, bias -2) -> eS fp16
       ctx psum [om*64+h, 0:64]=E^T.T@V, col 64 = Z (ones column)
     normalize on DVE: ctx = psum * (1/Z) + bv, DMA out.
"""

import numpy as np

import concourse.bass as bass
import concourse.bacc as bacc
import concourse.tile as tile
from concourse import mybir
from concourse import bass2jax

NCORES = 8
B, C, H, W = 16, 256, 64, 64
O = 256
PIX = H * W
BL = B // NCORES  # batches per core
SCALE = 1.0 / float(np.sqrt(32.0))
EXP_BIAS = -2.0  # softmax-invariant shift keeping exp() well inside fp16 range

FP32 = mybir.dt.float32
FP32R = mybir.dt.float32r
FP16 = mybir.dt.float16


def build_kernel(nc: bass.Bass):
    x_in = nc.declare_dram_parameter("x", [BL, C, PIX], FP16, isOutput=False)
    wT_in = nc.declare_dram_parameter("wT", [3, C, O], FP16, isOutput=False)
    bias_in = nc.declare_dram_parameter("bias", [3, O], FP32, isOutput=False)
    bv_in = nc.declare_dram_parameter("bv", [O], FP32, isOutput=False)
    out = nc.declare_dram_parameter("out", [BL, O, PIX], FP16, isOutput=True)

    with tile.TileContext(nc) as tc:
        with (
            tc.tile_pool(name="singles", bufs=1) as singles,
            tc.tile_pool(name="xin", bufs=2) as xpool,
            tc.tile_pool(name="p16", bufs=1) as p16pool,
            tc.tile_pool(name="tsp", bufs=2) as tpool,
            tc.tile_pool(name="small", bufs=8) as small,
            tc.tile_pool(name="psA", bufs=3, space="PSUM") as psA,
            tc.tile_pool(name="psS", bufs=2, space="PSUM") as psS,
            tc.tile_pool(name="psC", bufs=3, space="PSUM") as psC,
        ):
            # ---- constants loaded once ----
            w_sb = singles.tile([128, 3, 2, O], FP16)  # [c', proj, cc, o]
            nc.sync.dma_start(
                out=w_sb,
                in_=wT_in.rearrange("t (cc c) o -> c t cc o", cc=2),
            )
            bias_sb = singles.tile([128, 3, 2], FP32)  # [o', proj, oc]
            nc.sync.dma_start(
                out=bias_sb,
                in_=bias_in.rearrange("t (oc o) -> o t oc", oc=2),
            )
            # bv broadcast to [p=(om,h), j]
            bv_sb = singles.tile([128, 128], FP32)
            bv_ap = bv_in[:]
            bv_bcast = bass.AP(
                tensor=bv_ap.tensor,
                offset=bv_ap.offset,
                ap=[[128, 2], [0, 64], [1, 128]],
            )
            nc.sync.dma_start(out=bv_sb, in_=bv_bcast)
            expb_sb = singles.tile([128, 1], FP32)
            nc.vector.memset(expb_sb, EXP_BIAS)

            tensors = {}

            def emit_front(b):
                xsb = []
                for cc in range(2):
                    xt = xpool.tile([128, PIX], FP16, tag="xsb")
                    nc.sync.dma_start(out=xt, in_=x_in[b, cc * 128 : (cc + 1) * 128, :])
                    xsb.append(xt)

                q16 = p16pool.tile([128, H, 2, W], FP16, tag="q16")  # [j, h, om, w]
                k16 = p16pool.tile([128, H, 2, W], FP16, tag="k16")
                v16 = p16pool.tile([128, W, 2, H], FP16, tag="v16")  # [j, w, om, g]

                for proj in range(3):
                    for oc in range(2):
                        for nt in range(8):
                            ps = psA.tile([128, 512], FP32, tag="ps_proj")
                            for cc in range(2):
                                nc.tensor.matmul(
                                    ps,
                                    lhsT=w_sb[:, proj, cc, oc * 128 : (oc + 1) * 128],
                                    rhs=xsb[cc][:, nt * 512 : (nt + 1) * 512],
                                    start=(cc == 0),
                                    stop=(cc == 1),
                                )
                            bias_ap = bias_sb[:, proj, oc : oc + 1]
                            if proj == 0:  # q
                                nc.scalar.activation(
                                    out=q16[:, nt * 8 : (nt + 1) * 8, oc, :],
                                    in_=ps.rearrange("p (h w) -> p h w", w=W),
                                    func=mybir.ActivationFunctionType.Identity,
                                    bias=bias_ap,
                                    scale=1.0,
                                )
                            elif proj == 1:  # k
                                nc.vector.tensor_scalar_add(
                                    out=k16[:, nt * 8 : (nt + 1) * 8, oc, :],
                                    in0=ps.rearrange("p (h w) -> p h w", w=W),
                                    scalar1=bias_ap,
                                )
                            else:  # v
                                nc.vector.tensor_scalar_add(
                                    out=v16[:, :, oc, nt * 8 : (nt + 1) * 8].rearrange(
                                        "p w g -> p g w"
                                    ),
                                    in0=ps.rearrange("p (g w) -> p g w", w=W),
                                    scalar1=bias_ap,
                                )

                qS = tpool.tile([128, H, 128], FP16, tag="qS")  # [om*64+w, h, j]
                kS = tpool.tile([128, H, 128], FP16, tag="kS")
                vS = tpool.tile([128, W + 1, 128], FP16, tag="vS")
                nc.vector.memset(vS[:, W, :], 1.0)
                for h in range(H):
                    nc.sync.dma_start_transpose(
                        out=kS[:, h, :], in_=k16[:, h, :, :].rearrange("p a b -> p (a b)")
                    )
                    nc.sync.dma_start_transpose(
                        out=qS[:, h, :], in_=q16[:, h, :, :].rearrange("p a b -> p (a b)")
                    )
                for h in range(H):
                    nc.sync.dma_start_transpose(
                        out=vS[:, h, :], in_=v16[:, h, :, :].rearrange("p a b -> p (a b)")
                    )
                tensors[b] = (qS, kS, vS)

            def emit_attn(b):
                qS, kS, vS = tensors[b]
                JG = 8
                PG = 4
                for jg in range(0, 128, JG):
                    oc8 = small.tile([128, JG, W], FP16, tag="oc8")
                    sp8f = psS.tile([128, 512], FP32, tag="sp8")
                    sp8 = sp8f.rearrange("p (i h) -> p i h", h=H)
                    for i in range(JG):
                        j = jg + i
                        for om in range(2):
                            pr = slice(om * 64, om * 64 + 64)
                            nc.tensor.matmul(
                                sp8[pr, i, :],
                                lhsT=kS[pr, :, j],
                                rhs=qS[pr, :, j],
                                start=True,
                                stop=True,
                            )
                    eS8 = small.tile([128, JG, H], FP16, tag="eS8")
                    nc.scalar.activation(
                        out=eS8,
                        in_=sp8,
                        func=mybir.ActivationFunctionType.Exp,
                        bias=expb_sb,
                        scale=1.0,
                    )
                    for sg in range(jg, jg + JG, PG):
                        cp4f = psC.tile([128, 512], FP32, tag="cp4")
                        cp4 = cp4f[:, 0 : PG * (W + 1)].rearrange(
                            "p (i c) -> p i c", c=W + 1
                        )
                        for i in range(PG):
                            j = sg + i
                            for om in range(2):
                                pr = slice(om * 64, om * 64 + 64)
                                nc.tensor.matmul(
                                    cp4[pr, i, :],
                                    lhsT=eS8[pr, j - jg, :],
                                    rhs=vS[pr, :, j],
                                    start=True,
                                    stop=True,
                                )
                        rz4 = small.tile([128, PG], FP32, tag="rz4")
                        nc.vector.reciprocal(out=rz4, in_=cp4[:, :, W])
                        for i in range(PG):
                            j = sg + i
                            if j % 2 == 0:
                                nc.scalar.activation(
                                    out=oc8[:, j - jg, :],
                                    in_=cp4[:, i, 0:W],
                                    func=mybir.ActivationFunctionType.Identity,
                                    bias=bv_sb[:, j : j + 1],
                                    scale=rz4[:, i : i + 1],
                                )
                            else:
                                nc.vector.tensor_scalar(
                                    out=oc8[:, j - jg, :],
                                    in0=cp4[:, i, 0:W],
                                    scalar1=rz4[:, i : i + 1],
                                    scalar2=bv_sb[:, j : j + 1],
                                    op0=mybir.AluOpType.mult,
                                    op1=mybir.AluOpType.add,
                                )
                    for om in range(2):
                        nc.sync.dma_start(
                            out=out[b, om * 128 + jg : om * 128 + jg + JG, :].rearrange(
                                "j (h w) -> h j w", w=W
                            ),
                            in_=oc8[om * 64 : om * 64 + 64, :, :],
                        )

            for b in range(BL):
                emit_front(b)
            for b in range(BL):
                emit_attn(b)
    return nc


_NC_CACHE = {}


def get_nc():
    if "nc" not in _NC_CACHE:
        nc = bacc.Bacc(None, target_bir_lowering=False)
        build_kernel(nc)
        nc.finalize()
        _NC_CACHE["nc"] = nc
    return _NC_CACHE["nc"]


def prep_in_maps(x, wq, bq, wk, bk, wv, bv):
    wT = np.stack(
        [
            np.ascontiguousarray((wq * SCALE).T),
            np.ascontiguousarray(wk.T),
            np.ascontiguousarray(wv.T),
        ]
    ).astype(np.float16)
    biases = np.stack([bq * SCALE, bk, np.zeros_like(bv)]).astype(np.float32)
    # note: bv is folded in at the output stage, not in the v projection
    xs = np.ascontiguousarray(x.reshape(NCORES, BL, C, PIX)).astype(np.float16)
    bv32 = np.ascontiguousarray(bv).astype(np.float32)
    return [
        {"x": xs[i], "wT": wT, "bias": biases, "bv": bv32} for i in range(NCORES)
    ]


def kernel(x, wq, bq, wk, bk, wv, bv):
    nc = get_nc()
    in_maps = prep_in_maps(x, wq, bq, wk, bk, wv, bv)
    results = bass2jax.run_bass_via_pjrt(nc, in_maps, n_cores=NCORES)
    outs = [np.asarray(r["out"]).reshape(BL, O, H, W) for r in results]
    return np.concatenate(outs, axis=0).astype(np.float32)



# revision 4
# speedup vs baseline: 1.8111x; 1.8111x over previous
"""Trainium2 Bass kernel for nn_MultiHeadedSelfAttention_5179730559275.

Reference math (per batch b):
  q = wq @ x + bq ; k = wk @ x + bk ; v = wv @ x + bv        (1x1 conv, C=256 -> O=256)
  per o-channel (o = om*128 + j), with Q_o,K_o,V_o = 64x64 images [H,W]:
    S_o = Q_o @ K_o^T / sqrt(32); P_o = softmax(S_o, axis=-1); ctx_o = P_o @ V_o

Sharding: data-parallel over batch, 2 batches per core on 8 cores.

Per-core pipeline (per batch):
  1. Projection on PE (lhsT = w^T fp16 stationary, rhs = x fp16 moving,
     n=512) -> psum [o', 512]; ACT evacuates psum + bias -> fp16 SBUF
     (q/k in streaming 8-row chunks [j, 8h, om, w]; v full [j, om, g, w]).
     bv is folded into the v projection (softmax rows sum to 1).
  2. PE transposes (matmul vs identity) re-lay per-channel images with
     spatial on partitions, pairing channels o and o+128:
       qS/kS: [om*64+w, j, h]   vS: [om*64+g, j, (w|ones)]
     4 transposes share one psum bank; one DVE/ACT op evacuates each.
  3. Attention per pair j: quadrant matmuls (K=64 at partition bases 0/64):
       S^T psum [om*64+g, h] ; exp (ACT, bias -2) -> eS fp16
       ctx psum [om*64+h, 0:64]=E^T.T@V, col 64 = Z (ones column)
     DVE: rz = 1/Z, ctx = psum * rz (bv already in V), DMA out.
  Software pipelining: q/k transposes lag projection by one chunk; ctx
  matmuls lag score matmuls by one group; phase order is
  front(0), attn(0), front(1), attn(1).
"""

import numpy as np

import concourse.bass as bass
import concourse.bacc as bacc
import concourse.tile as tile
from concourse import mybir
from concourse import bass2jax
from concourse.masks import make_identity

NCORES = 8
B, C, H, W = 16, 256, 64, 64
O = 256
PIX = H * W
BL = B // NCORES  # batches per core
SCALE = 1.0 / float(np.sqrt(32.0))
EXP_BIAS = -2.0  # softmax-invariant shift keeping exp() well inside fp16 range

FP32 = mybir.dt.float32
FP16 = mybir.dt.float16


def build_kernel(nc: bass.Bass):
    x_in = nc.declare_dram_parameter("x", [BL, C, PIX], FP16, isOutput=False)
    wT_in = nc.declare_dram_parameter("wT", [3, C, O], FP16, isOutput=False)
    bias_in = nc.declare_dram_parameter("bias", [3, O], FP32, isOutput=False)
    out = nc.declare_dram_parameter("out", [BL, O, PIX], FP16, isOutput=True)

    with tile.TileContext(nc) as tc:
        with (
            tc.tile_pool(name="singles", bufs=1) as singles,
            tc.tile_pool(name="xin", bufs=4) as xpool,
            tc.tile_pool(name="chunks", bufs=2) as chpool,
            tc.tile_pool(name="vfull", bufs=2) as vpool,
            tc.tile_pool(name="tsp", bufs=2) as tpool,
            tc.tile_pool(name="small", bufs=3) as small,
            tc.tile_pool(name="psA", bufs=2, space="PSUM") as psA,
            tc.tile_pool(name="psT", bufs=2, space="PSUM") as psT,
            tc.tile_pool(name="psS", bufs=2, space="PSUM") as psS,
            tc.tile_pool(name="psC", bufs=2, space="PSUM") as psC,
        ):
            # ---- constants loaded once ----
            w_sb = singles.tile([128, 3, 2, O], FP16)  # [c', proj, cc, o]
            nc.sync.dma_start(
                out=w_sb,
                in_=wT_in.rearrange("t (cc c) o -> c t cc o", cc=2),
            )
            bias_sb = singles.tile([128, 3, 2], FP32)  # [o', proj, oc]
            nc.sync.dma_start(
                out=bias_sb,
                in_=bias_in.rearrange("t (oc o) -> o t oc", oc=2),
            )
            expb_sb = singles.tile([128, 1], FP32)
            nc.vector.memset(expb_sb, EXP_BIAS)
            ident = singles.tile([128, 128], FP16)
            make_identity(nc, ident)

            # x tiles for both batches prefetched (sync queue, overlaps)
            xsb = {}
            for b in range(BL):
                for cc in range(2):
                    xt = xpool.tile([128, PIX], FP16, tag="xsb")
                    nc.sync.dma_start(
                        out=xt, in_=x_in[b, cc * 128 : (cc + 1) * 128, :]
                    )
                    xsb[(b, cc)] = xt

            tensors = {}

            def emit_front(b):
                # qS/kS: [om*64+w, j, h]; vS: [om*64+g, j, w|ones]
                qS = tpool.tile([128, 128, H], FP16, tag="qS")
                kS = tpool.tile([128, 128, H], FP16, tag="kS")
                vS = tpool.tile([128, 128, W + 1], FP16, tag="vS")
                nc.gpsimd.memset(vS[:, :, W], 1.0)

                # ---- q, k: streamed chunks + lagged PE transposes ----
                for proj, dst in ((0, qS), (1, kS)):
                    pending = None  # (chunk, nt) awaiting transpose
                    for nt in range(8):
                        ch = chpool.tile(
                            [128, 8, 2, W], FP16, tag=f"ch{proj}"
                        )  # [j, h', oc, w]
                        for oc in range(2):
                            ps = psA.tile([128, 512], FP32, tag="psA")
                            for cc in range(2):
                                nc.tensor.matmul(
                                    ps,
                                    lhsT=w_sb[:, proj, cc, oc * 128 : (oc + 1) * 128],
                                    rhs=xsb[(b, cc)][:, nt * 512 : (nt + 1) * 512],
                                    start=(cc == 0),
                                    stop=(cc == 1),
                                )
                            nc.scalar.activation(
                                out=ch[:, :, oc, :],
                                in_=ps.rearrange("p (h w) -> p h w", w=W),
                                func=mybir.ActivationFunctionType.Identity,
                                bias=bias_sb[:, proj, oc : oc + 1],
                                scale=1.0,
                            )
                        if pending is not None:
                            _emit_qk_transpose(pending[0], pending[1], dst)
                        pending = (ch, nt)
                    _emit_qk_transpose(pending[0], pending[1], dst)

                # ---- v: full projection (bias = bv), then PE transposes ----
                v16 = vpool.tile([128, 2, H, W], FP16, tag="v16")  # [j, oc, g, w]
                for oc in range(2):
                    for nt in range(8):
                        ps = psA.tile([128, 512], FP32, tag="psA")
                        for cc in range(2):
                            nc.tensor.matmul(
                                ps,
                                lhsT=w_sb[:, 2, cc, oc * 128 : (oc + 1) * 128],
                                rhs=xsb[(b, cc)][:, nt * 512 : (nt + 1) * 512],
                                start=(cc == 0),
                                stop=(cc == 1),
                            )
                        nc.scalar.activation(
                            out=v16[:, oc, nt * 8 : (nt + 1) * 8, :],
                            in_=ps.rearrange("p (g w) -> p g w", w=W),
                            func=mybir.ActivationFunctionType.Identity,
                            bias=bias_sb[:, 2, oc : oc + 1],
                            scale=1.0,
                        )
                for wg in range(16):
                    pt = psT.tile([128, 4, 128], FP16, tag="psT")
                    for i in range(4):
                        w = wg * 4 + i
                        nc.tensor.transpose(
                            pt[:, i, :],
                            v16[:, :, :, w].rearrange("p a b -> p (a b)"),
                            ident,
                        )
                    nc.vector.tensor_copy(
                        out=vS[:, :, wg * 4 : (wg + 1) * 4].rearrange(
                            "p j w -> p w j"
                        ),
                        in_=pt,
                    )
                tensors[b] = (qS, kS, vS)

            def _emit_qk_transpose(ch, nt, dst):
                # 8 h-rows -> 2 psum groups of 4 -> evac to dst[:, :, h0:h0+4]
                for g in range(2):
                    pt = psT.tile([128, 4, 128], FP16, tag="psT")
                    for i in range(4):
                        nc.tensor.transpose(
                            pt[:, i, :],
                            ch[:, g * 4 + i, :, :].rearrange("p a b -> p (a b)"),
                            ident,
                        )
                    h0 = nt * 8 + g * 4
                    # alternate evac engine to balance ACT/DVE load
                    op = nc.vector if (nt + g) % 3 else nc.scalar
                    if op is nc.vector:
                        nc.vector.tensor_copy(
                            out=dst[:, :, h0 : h0 + 4].rearrange("p j h -> p h j"),
                            in_=pt,
                        )
                    else:
                        nc.scalar.activation(
                            out=dst[:, :, h0 : h0 + 4].rearrange("p j h -> p h j"),
                            in_=pt,
                            func=mybir.ActivationFunctionType.Identity,
                        )

            def emit_attn(b):
                qS, kS, vS = tensors[b]
                JG = 8
                state = None  # (jg, eS8) pending ctx stage

                def emit_ctx(jg, eS8):
                    oc8 = small.tile([128, JG, W], FP16, tag="oc8")
                    for sg in range(2):
                        cp4f = psC.tile([128, 512], FP32, tag="psC")
                        cp4 = cp4f[:, 0 : 4 * (W + 1)].rearrange(
                            "p (i c) -> p i c", c=W + 1
                        )
                        for i in range(4):
                            j = jg * JG + sg * 4 + i
                            for om in range(2):
                                pr = slice(om * 64, om * 64 + 64)
                                nc.tensor.matmul(
                                    cp4[pr, i, :],
                                    lhsT=eS8[pr, sg * 4 + i, :],
                                    rhs=vS[pr, j, :],
                                    start=True,
                                    stop=True,
                                )
                        rz4 = small.tile([128, 4], FP32, tag="rz4")
                        nc.vector.reciprocal(out=rz4, in_=cp4[:, :, W])
                        nc.vector.tensor_tensor(
                            oc8[:, sg * 4 : (sg + 1) * 4, :],
                            cp4[:, :, 0:W],
                            rz4[:, :, None].to_broadcast([128, 4, W]),
                            mybir.AluOpType.mult,
                        )
                    for om in range(2):
                        j0 = jg * JG
                        nc.sync.dma_start(
                            out=out[
                                b, om * 128 + j0 : om * 128 + j0 + JG, :
                            ].rearrange("j (h w) -> h j w", w=W),
                            in_=oc8[om * 64 : om * 64 + 64, :, :],
                        )

                for jg in range(16):
                    sp8f = psS.tile([128, 512], FP32, tag="psS")
                    sp8 = sp8f.rearrange("p (i h) -> p i h", h=H)
                    for i in range(JG):
                        j = jg * JG + i
                        for om in range(2):
                            pr = slice(om * 64, om * 64 + 64)
                            nc.tensor.matmul(
                                sp8[pr, i, :],
                                lhsT=kS[pr, j, :],
                                rhs=qS[pr, j, :],
                                start=True,
                                stop=True,
                            )
                    eS8 = small.tile([128, JG, H], FP16, tag="eS8")
                    nc.scalar.activation(
                        out=eS8,
                        in_=sp8,
                        func=mybir.ActivationFunctionType.Exp,
                        bias=expb_sb,
                        scale=1.0,
                    )
                    if state is not None:
                        emit_ctx(*state)
                    state = (jg, eS8)
                emit_ctx(*state)

            emit_front(0)
            emit_attn(0)
            if BL > 1:
                emit_front(1)
                emit_attn(1)
    return nc


_NC_CACHE = {}


def get_nc():
    if "nc" not in _NC_CACHE:
        nc = bacc.Bacc(None, target_bir_lowering=False)
        build_kernel(nc)
        nc.finalize()
        _NC_CACHE["nc"] = nc
    return _NC_CACHE["nc"]


def prep_in_maps(x, wq, bq, wk, bk, wv, bv):
    wT = np.stack(
        [
            np.ascontiguousarray((wq * SCALE).T),
            np.ascontiguousarray(wk.T),
            np.ascontiguousarray(wv.T),
        ]
    ).astype(np.float16)
    # bv rides in the v projection: softmax rows sum to 1, so ctx = P@V0 + bv
    biases = np.stack([bq * SCALE, bk, bv]).astype(np.float32)
    xs = np.ascontiguousarray(x.reshape(NCORES, BL, C, PIX)).astype(np.float16)
    return [{"x": xs[i], "wT": wT, "bias": biases} for i in range(NCORES)]


def kernel(x, wq, bq, wk, bk, wv, bv):
    nc = get_nc()
    in_maps = prep_in_maps(x, wq, bq, wk, bk, wv, bv)
    results = bass2jax.run_bass_via_pjrt(nc, in_maps, n_cores=NCORES)
    outs = [np.asarray(r["out"]).reshape(BL, O, H, W) for r in results]
    return np.concatenate(outs, axis=0).astype(np.float32)


# revision 11
# speedup vs baseline: 3.1473x; 1.7378x over previous
"""Trainium2 Bass kernel for nn_MultiHeadedSelfAttention_5179730559275.

Reference math (per batch b):
  q = wq @ x + bq ; k = wk @ x + bk ; v = wv @ x + bv        (1x1 conv, C=256 -> O=256)
  per o-channel (o = om*128 + j), with Q_o,K_o,V_o = 64x64 images [H,W]:
    S_o = Q_o @ K_o^T / sqrt(32); P_o = softmax(S_o, axis=-1); ctx_o = P_o @ V_o

Sharding: data-parallel over batch, 2 batches per core on 8 cores.

Per-core pipeline (per batch):
  1. Projection on PE (lhsT = w^T fp16 stationary, rhs = x fp16 moving,
     n=512) -> psum [o', 512]; ACT/DVE evacuate psum + bias -> fp16 SBUF
     with fully contiguous destinations (q/k in streaming chunks
     [j, oc, 8h, w]; v full [j, oc, g, w]).  bv is folded into the v
     projection (softmax rows sum to 1, so ctx = P@V0 + bv).
  2. PE transposes (matmul vs identity, fp16 psum) re-lay per-channel
     images with spatial on partitions, pairing channels o and o+128:
       qS/kS: [om*64+w, h, j]   vS: [om*64+g, w|ones, j]
     8 transposes fill one fp16 psum bank [128, 8, 128]; a single
     contiguous [128, 1024] DVE op evacuates it (2x 16-bit mode).
  3. Attention per pair j: quadrant matmuls (K=64 at partition bases 0/64):
       S^T psum [om*64+g, h] ; exp (ACT, bias -2) -> eS fp16
       ctx psum [om*64+h, 0:64]=E^T.T@V, col 64 = Z (ones column)
     DVE: rz = 1/Z, ctx = psum * rz broadcast (bv already in V), DMA out.
  Software pipelining: q/k transposes lag projection by one chunk; ctx
  matmuls lag score matmuls by one group; phase order is
  front(0), attn(0), front(1), attn(1).
"""

import numpy as np

import concourse.bass as bass
import concourse.bacc as bacc
import concourse.tile as tile
from concourse import mybir
from concourse import bass2jax
from concourse.masks import make_identity

NCORES = 8
B, C, H, W = 16, 256, 64, 64
O = 256
PIX = H * W
BL = B // NCORES  # batches per core
SCALE = 1.0 / float(np.sqrt(32.0))
EXP_BIAS = -2.0  # softmax-invariant shift keeping exp() well inside fp16 range

FP32 = mybir.dt.float32
FP16 = mybir.dt.float16


def build_kernel(nc: bass.Bass):
    x_in = nc.declare_dram_parameter("x", [BL, C, PIX], FP16, isOutput=False)
    wT_in = nc.declare_dram_parameter("wT", [3, C, O], FP16, isOutput=False)
    bias_in = nc.declare_dram_parameter("bias", [3, O], FP32, isOutput=False)
    out = nc.declare_dram_parameter("out", [BL, O, PIX], FP16, isOutput=True)

    with tile.TileContext(nc) as tc:
        with (
            tc.tile_pool(name="singles", bufs=1) as singles,
            tc.tile_pool(name="xin", bufs=4) as xpool,
            tc.tile_pool(name="chunks", bufs=2) as chpool,
            tc.tile_pool(name="vfull", bufs=2) as vpool,
            tc.tile_pool(name="tsp", bufs=2) as tpool,
            tc.tile_pool(name="small", bufs=3) as small,
            tc.tile_pool(name="psA", bufs=2, space="PSUM") as psA,
            tc.tile_pool(name="psT", bufs=2, space="PSUM") as psT,
            tc.tile_pool(name="psS", bufs=2, space="PSUM") as psS,
            tc.tile_pool(name="psC", bufs=2, space="PSUM") as psC,
        ):
            # ---- constants loaded once ----
            w_sb = singles.tile([128, 3, 2, O], FP16)  # [c', proj, cc, o]
            nc.sync.dma_start(
                out=w_sb,
                in_=wT_in.rearrange("t (cc c) o -> c t cc o", cc=2),
            )
            bias_sb = singles.tile([128, 3, 2], FP32)  # [o', proj, oc]
            nc.sync.dma_start(
                out=bias_sb,
                in_=bias_in.rearrange("t (oc o) -> o t oc", oc=2),
            )
            expb_sb = singles.tile([128, 1], FP32)
            nc.vector.memset(expb_sb, EXP_BIAS)
            ident = singles.tile([128, 128], FP16)
            make_identity(nc, ident)

            # x tiles for both batches prefetched (sync queue, overlaps)
            xsb = {}
            for b in range(BL):
                for cc in range(2):
                    xt = xpool.tile([128, PIX], FP16, tag="xsb")
                    nc.sync.dma_start(
                        out=xt, in_=x_in[b, cc * 128 : (cc + 1) * 128, :]
                    )
                    xsb[(b, cc)] = xt

            tensors = {}
            evac1_ctr = [0]

            def evac1(dst, ps, proj, oc):
                # psum [o', 512] + bias -> fp16 SBUF, contiguous dest.
                # Round-robin 3:1 ACT:DVE to balance engine load.
                evac1_ctr[0] += 1
                if evac1_ctr[0] % 4 == 0:
                    nc.vector.tensor_scalar_add(
                        out=dst,
                        in0=ps.rearrange("p (h w) -> p h w", w=W),
                        scalar1=bias_sb[:, proj, oc : oc + 1],
                    )
                else:
                    nc.scalar.activation(
                        out=dst,
                        in_=ps.rearrange("p (h w) -> p h w", w=W),
                        func=mybir.ActivationFunctionType.Identity,
                        bias=bias_sb[:, proj, oc : oc + 1],
                        scale=1.0,
                    )

            def emit_front(b):
                # qS/kS: [om*64+w, h, j]; vS: [om*64+g, w|ones, j]
                qS = tpool.tile([128, H, 128], FP16, tag="qS")
                kS = tpool.tile([128, H, 128], FP16, tag="kS")
                vS = tpool.tile([128, W + 1, 128], FP16, tag="vS")
                nc.gpsimd.memset(vS[:, W, :], 1.0)

                # ---- q, k: streamed chunks + lagged PE transposes ----
                for proj, dst in ((0, qS), (1, kS)):
                    pending = None  # (chunk, nt) awaiting transpose
                    for nt in range(8):
                        ch = chpool.tile(
                            [128, 8, 2, W], FP16, tag=f"ch{proj}"
                        )  # [j, h', oc, w]
                        for oc in range(2):
                            ps = psA.tile([128, 512], FP32, tag="psA")
                            for cc in range(2):
                                nc.tensor.matmul(
                                    ps,
                                    lhsT=w_sb[:, proj, cc, oc * 128 : (oc + 1) * 128],
                                    rhs=xsb[(b, cc)][:, nt * 512 : (nt + 1) * 512],
                                    start=(cc == 0),
                                    stop=(cc == 1),
                                )
                            evac1(ch[:, :, oc, :], ps, proj, oc)
                        if pending is not None:
                            _emit_qk_transpose(pending[0], pending[1], dst)
                        pending = (ch, nt)
                    _emit_qk_transpose(pending[0], pending[1], dst)

                # ---- v: full projection (bias = bv), then PE transposes ----
                v16 = vpool.tile([128, 2, H, W], FP16, tag="v16")  # [j, oc, g, w]
                for oc in range(2):
                    for nt in range(8):
                        ps = psA.tile([128, 512], FP32, tag="psA")
                        for cc in range(2):
                            nc.tensor.matmul(
                                ps,
                                lhsT=w_sb[:, 2, cc, oc * 128 : (oc + 1) * 128],
                                rhs=xsb[(b, cc)][:, nt * 512 : (nt + 1) * 512],
                                start=(cc == 0),
                                stop=(cc == 1),
                            )
                        evac1(v16[:, oc, nt * 8 : (nt + 1) * 8, :], ps, 2, oc)
                for vg in range(8):
                    pt = psT.tile([128, 8, 128], FP16, tag="psT")
                    for i in range(8):
                        w = vg * 8 + i
                        # [j, (oc, g)] -> [(oc, g), j] per w column
                        nc.tensor.transpose(
                            pt[:, i, :],
                            v16[:, :, :, w].rearrange("p a b -> p (a b)"),
                            ident,
                        )
                    nc.vector.tensor_copy(
                        out=vS[:, vg * 8 : (vg + 1) * 8, :], in_=pt
                    )
                tensors[b] = (qS, kS, vS)

            def _emit_qk_transpose(ch, nt, dst):
                # 8 h-rows -> one fp16 psum bank -> one contiguous evac
                pt = psT.tile([128, 8, 128], FP16, tag="psT")
                for i in range(8):
                    # [j, (oc, w)] -> [(oc, w), j] per h row
                    nc.tensor.transpose(
                        pt[:, i, :],
                        ch[:, i, :, :].rearrange("p a b -> p (a b)"),
                        ident,
                    )
                nc.vector.tensor_copy(
                    out=dst[:, nt * 8 : (nt + 1) * 8, :], in_=pt
                )

            def emit_attn(b):
                qS, kS, vS = tensors[b]
                JG = 8
                state = None  # (jg, eS8) pending ctx stage

                def emit_ctx(jg, eS8):
                    oc8 = small.tile([128, JG, W], FP16, tag="oc8")
                    for sg in range(2):
                        cp4f = psC.tile([128, 512], FP32, tag="psC")
                        cp4 = cp4f[:, 0 : 4 * (W + 1)].rearrange(
                            "p (i c) -> p i c", c=W + 1
                        )
                        for i in range(4):
                            j = jg * JG + sg * 4 + i
                            for om in range(2):
                                pr = slice(om * 64, om * 64 + 64)
                                nc.tensor.matmul(
                                    cp4[pr, i, :],
                                    lhsT=eS8[pr, sg * 4 + i, :],
                                    rhs=vS[pr, :, j],
                                    start=True,
                                    stop=True,
                                )
                        rz4 = small.tile([128, 4], FP32, tag="rz4")
                        nc.vector.reciprocal(out=rz4, in_=cp4[:, :, W])
                        nc.vector.tensor_tensor(
                            oc8[:, sg * 4 : (sg + 1) * 4, :],
                            cp4[:, :, 0:W],
                            rz4[:, :, None].to_broadcast([128, 4, W]),
                            mybir.AluOpType.mult,
                        )
                    for om in range(2):
                        j0 = jg * JG
                        nc.sync.dma_start(
                            out=out[
                                b, om * 128 + j0 : om * 128 + j0 + JG, :
                            ].rearrange("j (h w) -> h j w", w=W),
                            in_=oc8[om * 64 : om * 64 + 64, :, :],
                        )

                for jg in range(16):
                    sp8f = psS.tile([128, 512], FP32, tag="psS")
                    sp8 = sp8f.rearrange("p (i h) -> p i h", h=H)
                    for i in range(JG):
                        j = jg * JG + i
                        for om in range(2):
                            pr = slice(om * 64, om * 64 + 64)
                            nc.tensor.matmul(
                                sp8[pr, i, :],
                                lhsT=kS[pr, :, j],
                                rhs=qS[pr, :, j],
                                start=True,
                                stop=True,
                            )
                    eS8 = small.tile([128, JG, H], FP16, tag="eS8")
                    nc.scalar.activation(
                        out=eS8,
                        in_=sp8,
                        func=mybir.ActivationFunctionType.Exp,
                        bias=expb_sb,
                        scale=1.0,
                    )
                    if state is not None:
                        emit_ctx(*state)
                    state = (jg, eS8)
                emit_ctx(*state)

            emit_front(0)
            emit_attn(0)
            if BL > 1:
                emit_front(1)
                emit_attn(1)
    return nc


_NC_CACHE = {}


def get_nc():
    if "nc" not in _NC_CACHE:
        nc = bacc.Bacc(None, target_bir_lowering=False)
        build_kernel(nc)
        nc.finalize()
        _NC_CACHE["nc"] = nc
    return _NC_CACHE["nc"]


def prep_in_maps(x, wq, bq, wk, bk, wv, bv):
    wT = np.stack(
        [
            np.ascontiguousarray((wq * SCALE).T),
            np.ascontiguousarray(wk.T),
            np.ascontiguousarray(wv.T),
        ]
    ).astype(np.float16)
    # bv rides in the v projection: softmax rows sum to 1, so ctx = P@V0 + bv
    biases = np.stack([bq * SCALE, bk, bv]).astype(np.float32)
    xs = np.ascontiguousarray(x.reshape(NCORES, BL, C, PIX)).astype(np.float16)
    return [{"x": xs[i], "wT": wT, "bias": biases} for i in range(NCORES)]


def kernel(x, wq, bq, wk, bk, wv, bv):
    nc = get_nc()
    in_maps = prep_in_maps(x, wq, bq, wk, bk, wv, bv)
    results = bass2jax.run_bass_via_pjrt(nc, in_maps, n_cores=NCORES)
    outs = [np.asarray(r["out"]).reshape(BL, O, H, W) for r in results]
    return np.concatenate(outs, axis=0).astype(np.float32)
